# revision 3
# baseline (speedup 1.0000x reference)
"""Trainium2 Bass kernel for nn_DeepModel_70703751626759 (deep-BSDE forward sim).

Data-parallel over 8 NeuronCores: 32768 samples -> 4096/core -> 8 column
blocks of 512 samples. Feature-major layout with 32-row "slots": block
(p, q) lives at partitions [32p, 32p+32) and free columns [512q, 512(q+1))
of [128, 1024] tensors (p in 0..3, q in 0..1, block index j = p + 4q).

Per-slot state: XY = [X(16); Y(16)], ZU = [u(8); Zv(16); dH(8)].
The t-dependence of the MLP inputs is folded into per-step layer-1 biases.
dH's u/Zv dependence is recomposed through h2 (linearity), the multiplicative
noise dw * (.) is realized by scaling inputs (X, u, Zv) with a PE-broadcast
dw row and feeding the scaled copies back through matmuls. Losses are
reduced on-device via activation/STT accum_out; final tiny reductions and
the 8-core combine happen on host.
"""

import sys
import numpy as np

if "/opt/trn_rl_repo" not in sys.path:
    sys.path.insert(0, "/opt/trn_rl_repo")

N = 16
M = 8
T = 50
DT = 0.01
GAMMA = 0.1
SIGMA = 0.2
TAU = 0.5
H = 10
BATCH = 32768
NCORES = 8
CB = BATCH // NCORES      # 4096 samples per core
NB = 8                    # column blocks per core
BK = 512                  # samples per block
CHUNK = 13                # dw steps per DMA chunk
NCHUNK = 4                # ceil(50 / 13)

F32 = np.float32

# consts image column offsets (all lhsT blocks replicated on partition
# groups {0,32,64,96}; biases are [128, 1] columns)
C_W1 = 0
C_W2 = 32
C_W3 = 64
C_WZY = 96
C_WUXY = 128
C_WUZU = 160
C_WUSA = 192
C_WUSB = 224
C_WE = 256
C_WY1 = 288
C_WY2 = 320
C_WY3 = 352
C_WYP = 384
C_ONES = 416
C_SIG = 448
C_B1 = 480            # 64 columns (t = 0..49 used)
C_B2 = 544
C_B3 = 545
C_BXY = 546
C_BY1 = 547
C_BY2 = 548
C_BYI = 549
C_COLS = 640


def _ct(t):
    w = 1.0 if (t == 0 or t == T - 1) else 2.0
    return 0.5 * DT * w * TAU * TAU


def pack_weights(inp):
    """Build the [128, C_COLS] consts image (fp32)."""
    A = np.asarray(inp["A"], F32)
    Bm = np.asarray(inp["Bmat"], F32)
    C = np.asarray(inp["Cmat"], F32)
    D = np.asarray(inp["Dmat"], F32)
    ZW1, Zb1 = np.asarray(inp["Z_W1"], F32), np.asarray(inp["Z_b1"], F32)
    ZW2, Zb2 = np.asarray(inp["Z_W2"], F32), np.asarray(inp["Z_b2"], F32)
    ZW3, Zb3 = np.asarray(inp["Z_W3"], F32), np.asarray(inp["Z_b3"], F32)
    PW1, Pb1 = np.asarray(inp["phi_W1"], F32), np.asarray(inp["phi_b1"], F32)
    PW2, Pb2 = np.asarray(inp["phi_W2"], F32), np.asarray(inp["phi_b2"], F32)
    PW3, Pb3 = np.asarray(inp["phi_W3"], F32), np.asarray(inp["phi_b3"], F32)
    YW1, Yb1 = np.asarray(inp["Y0_W1"], F32), np.asarray(inp["Y0_b1"], F32)
    YW2, Yb2 = np.asarray(inp["Y0_W2"], F32), np.asarray(inp["Y0_b2"], F32)
    YW3, Yb3 = np.asarray(inp["Y0_W3"], F32), np.asarray(inp["Y0_b3"], F32)
    I16 = np.eye(16, dtype=F32)

    def blk(rows, cols):
        return np.zeros((rows, cols), F32)

    # mm1: rhs = XY slot [X;Y], out = h1pre [h1z(10); h1phi(10); 0(12)]
    W1R = blk(32, 32)
    W1R[0:16, 0:10] = ZW1[1:, :]
    W1R[0:16, 10:20] = PW1[1:, :]

    # mm2: rhs = H1 slot, out = h2pre
    W2R = blk(32, 32)
    W2R[0:10, 0:10] = ZW2
    W2R[10:20, 10:20] = PW2

    # mm3: rhs = H2 slot, out = [u(8); Zv(16); dH_h2_part(8)]
    W3R = blk(32, 32)
    W3R[10:20, 0:8] = PW3                 # u = phi_W3.T @ h2phi
    W3R[0:10, 8:24] = ZW3                 # Zv = Z_W3.T @ h2z
    W3R[0:10, 24:32] = ZW3 @ D            # (D.T @ Z3) part of dH
    W3R[10:20, 24:32] = PW3               # u part of dH

    # zuY: rhs = XY slot, out accumulates into [u; Zv; dH]
    WZY = blk(32, 32)
    WZY[16:32, 24:32] = Bm                # dH += Bmat.T @ Y

    # upd ch XY: rhs = XY slot, out = [Xn(16); Yn(16)]
    WUXY = blk(32, 32)
    WUXY[0:16, 0:16] = I16 + DT * A.T     # Xn: (I + DT A) @ X
    WUXY[0:16, 16:32] = -DT * I16         # Yn: -DT X
    WUXY[16:32, 16:32] = I16 - DT * A     # Yn: (I - DT A.T) @ Y

    # upd ch ZU: rhs = ZU slot [u; Zv; dH]
    WUZU = blk(32, 32)
    WUZU[0:8, 0:16] = DT * Bm.T           # Xn: DT B @ u
    WUZU[8:24, 16:32] = -DT * C           # Yn: -DT C.T @ Zv

    # upd ch SCA: rhs = dw*XY slot [Xt(16); junk(16)]
    WUSA = blk(32, 32)
    WUSA[0:16, 0:16] = C.T                # Xn: C @ (dw*X)

    # upd ch SCB: rhs = dw*ZU slot [ut(8); Zvt(16); junk(8)]
    WUSB = blk(32, 32)
    WUSB[0:8, 0:16] = D.T                 # Xn: D @ (dw*u)
    WUSB[8:24, 16:32] = I16               # Yn: dw*Zv

    # final error: rhs = XY slot, out = [Y - X (16); 0(16)]
    WE = blk(32, 32)
    WE[0:16, 0:16] = -I16
    WE[16:32, 0:16] = I16

    # Y0 init MLP
    WY1 = blk(32, 32)
    WY1[0:16, 0:10] = YW1
    WY2 = blk(32, 32)
    WY2[0:10, 0:10] = YW2
    WY3 = blk(32, 32)
    WY3[0:10, 16:32] = YW3                # Y-init into cols 16:32
    WYP = blk(32, 32)
    WYP[0:16, 0:16] = I16                 # X pass-through during init

    img = np.zeros((128, C_COLS), F32)
    reps = [(C_W1, W1R), (C_W2, W2R), (C_W3, W3R), (C_WZY, WZY),
            (C_WUXY, WUXY), (C_WUZU, WUZU), (C_WUSA, WUSA), (C_WUSB, WUSB),
            (C_WE, WE), (C_WY1, WY1), (C_WY2, WY2), (C_WY3, WY3),
            (C_WYP, WYP)]
    for p in range(4):
        for off, w in reps:
            img[32 * p: 32 * p + 32, off: off + 32] = w

    img[:, C_ONES: C_ONES + 32] = 1.0
    img[:, C_SIG: C_SIG + 16] = SIGMA     # sigma row: cols 0:16 of M

    # per-step layer-1 bias (t folded in) at rows r<20 of each slot
    for t in range(T):
        tv = F32(t * DT)
        b = np.concatenate([Zb1 + tv * ZW1[0, :], Pb1 + tv * PW1[0, :]])
        for p in range(4):
            img[32 * p: 32 * p + 20, C_B1 + t] = b

    b2 = np.concatenate([Zb2, Pb2])
    b3 = np.concatenate([Pb3, Zb3, Pb3 + D.T @ Zb3])
    for p in range(4):
        img[32 * p: 32 * p + 20, C_B2] = b2
        img[32 * p: 32 * p + 32, C_B3] = b3
        img[32 * p: 32 * p + 16, C_BXY] = GAMMA * DT
        img[32 * p: 32 * p + 10, C_BY1] = Yb1
        img[32 * p: 32 * p + 10, C_BY2] = Yb2
        img[32 * p + 16: 32 * p + 32, C_BYI] = Yb3
    return img


def pack_x0(X0, core):
    """[128, 1024] slot image of this core's X0 (Y rows zero)."""
    out = np.zeros((128, 1024), F32)
    base = core * CB
    for p in range(4):
        for q in range(2):
            j = p + 4 * q
            blkdata = X0[base + BK * j: base + BK * (j + 1), :]   # [512, 16]
            out[32 * p: 32 * p + 16, 512 * q: 512 * (q + 1)] = blkdata.T
    return np.ascontiguousarray(out)


def pack_dw(dw, core):
    """[4, NCHUNK*CHUNK*2*512] chunked dw rows for the PE broadcast."""
    base = core * CB
    out = np.zeros((4, NCHUNK * CHUNK * 2 * BK), F32)
    for c in range(NCHUNK):
        for toff in range(CHUNK):
            t = c * CHUNK + toff
            if t >= T:
                break
            for p in range(4):
                for q in range(2):
                    j = p + 4 * q
                    o = c * (CHUNK * 2 * BK) + (toff * 2 + q) * BK
                    out[p, o: o + BK] = dw[t, base + BK * j: base + BK * (j + 1), 0]
    return np.ascontiguousarray(out)


# ---------------------------------------------------------------------------
# numpy emulation of the exact device program (debug / algebra check)
# ---------------------------------------------------------------------------

def emulate_core(img, x0p, dwp, t_steps=T):
    def slots(x):
        # [128, 1024] -> list of 8 [32, 512] views, j = p + 4q
        return [x[32 * (j % 4): 32 * (j % 4) + 32,
                  512 * (j // 4): 512 * (j // 4) + 512] for j in range(NB)]

    def wk(off):
        return img[0:32, off: off + 32]

    def bias(col):
        return img[:, col: col + 1]

    XY = x0p.copy()
    lacc = np.zeros((128, 64), F32)
    eacc = np.zeros((128, 1), F32)

    # init Y0
    ph = np.zeros((128, 1024), F32)
    for j, s in enumerate(slots(XY)):
        sl = slots(ph)[j]
        sl[:] = wk(C_WY1).T @ s
    H1 = np.tanh(ph + bias(C_BY1))
    ph2 = np.zeros_like(ph)
    for j in range(NB):
        slots(ph2)[j][:] = wk(C_WY2).T @ slots(H1)[j]
    H2 = np.tanh(ph2 + bias(C_BY2))
    pu = np.zeros_like(ph)
    for j in range(NB):
        slots(pu)[j][:] = wk(C_WYP).T @ slots(XY)[j] + wk(C_WY3).T @ slots(H2)[j]
    XY = pu + bias(C_BYI)

    for t in range(t_steps):
        c, toff = divmod(t, CHUNK)
        # dw broadcast
        pdwb = np.zeros((128, 1024), F32)
        for j in range(NB):
            p, q = j % 4, j // 4
            o = c * (CHUNK * 2 * BK) + (toff * 2 + q) * BK
            dwrow = dwp[p, o: o + BK]
            slots(pdwb)[j][:] = np.broadcast_to(dwrow, (32, BK))
        ph1 = np.zeros((128, 1024), F32)
        for j in range(NB):
            slots(ph1)[j][:] = wk(C_W1).T @ slots(XY)[j]
        H1 = np.tanh(ph1 + bias(C_B1 + t))
        ph2 = np.zeros_like(ph1)
        for j in range(NB):
            slots(ph2)[j][:] = wk(C_W2).T @ slots(H1)[j]
        H2 = np.tanh(ph2 + bias(C_B2))
        pzu = np.zeros_like(ph1)
        for j in range(NB):
            slots(pzu)[j][:] = (wk(C_W3).T @ slots(H2)[j]
                                + wk(C_WZY).T @ slots(XY)[j])
        ZU = pzu + bias(C_B3)
        SCA = XY * pdwb
        SCB = ZU * pdwb
        lacc[:, t: t + 1] = np.sum((_ct(t) * ZU) * ZU, axis=1, keepdims=True)
        pu = np.zeros_like(ph1)
        for j in range(NB):
            p, q = j % 4, j // 4
            o = c * (CHUNK * 2 * BK) + (toff * 2 + q) * BK
            dwrow = dwp[p: p + 1, o: o + BK]                      # [1, 512]
            slots(pu)[j][:] = (wk(C_WUXY).T @ slots(XY)[j]
                               + wk(C_WUZU).T @ slots(ZU)[j]
                               + wk(C_WUSA).T @ slots(SCA)[j]
                               + wk(C_WUSB).T @ slots(SCB)[j]
                               + img[0:1, C_SIG: C_SIG + 32].T @ dwrow)
        XY = pu + bias(C_BXY)

    pe = np.zeros((128, 1024), F32)
    for j in range(NB):
        slots(pe)[j][:] = wk(C_WE).T @ slots(XY)[j]
    eacc[:, 0:1] = np.sum(pe * pe, axis=1, keepdims=True)
    return lacc, eacc


def reduce_outputs(laccs, eaccs, t_steps=T):
    """Combine per-core [128, 64] lacc and [128, >=1] eacc into [2]."""
    dh_rows = np.zeros(128, bool)
    e_rows = np.zeros(128, bool)
    for p in range(4):
        dh_rows[32 * p + 24: 32 * p + 32] = True
        e_rows[32 * p: 32 * p + 16] = True
    lc = 0.0
    lb = 0.0
    for lacc, eacc in zip(laccs, eaccs):
        lc += float(np.sum(lacc[dh_rows, :t_steps], dtype=np.float64))
        lb += float(np.sum(eacc[e_rows, 0], dtype=np.float64))
    return np.array([lb / BATCH, lc / BATCH], F32)


# ---------------------------------------------------------------------------
# device program
# ---------------------------------------------------------------------------

_BUILT = {}


def build(t_steps=T):
    if t_steps in _BUILT:
        return _BUILT[t_steps]
    from contextlib import ExitStack
    import concourse.bass as bass
    import concourse.tile as tile
    from concourse import bacc, mybir

    f32 = mybir.dt.float32
    AF = mybir.ActivationFunctionType
    OP = mybir.AluOpType

    nc = bacc.Bacc("TRN2", target_bir_lowering=False, debug=False)
    dwp_d = nc.dram_tensor("dwp", [4, NCHUNK * CHUNK * 2 * BK], f32,
                           kind="ExternalInput").ap()
    x0p_d = nc.dram_tensor("x0p", [128, 1024], f32, kind="ExternalInput").ap()
    wpk_d = nc.dram_tensor("wpack", [128, C_COLS], f32,
                           kind="ExternalInput").ap()
    lacc_d = nc.dram_tensor("out_lacc", [128, 64], f32,
                            kind="ExternalOutput").ap()
    eacc_d = nc.dram_tensor("out_eacc", [128, 8], f32,
                            kind="ExternalOutput").ap()

    def SL(tens, p, q, rows=32, r0=0):
        return tens[32 * p + r0: 32 * p + r0 + rows, 512 * q: 512 * (q + 1)]

    with tile.TileContext(nc) as tc, ExitStack() as ctx:
        sb = ctx.enter_context(tc.tile_pool(name="sb", bufs=1))
        dwpool = ctx.enter_context(tc.tile_pool(name="dwp", bufs=2))
        ps = ctx.enter_context(tc.tile_pool(name="ps", bufs=1, space="PSUM"))

        ck = sb.tile([128, C_COLS], f32, tag="consts")
        nc.sync.dma_start(out=ck[:, :], in_=wpk_d[:, :])
        XY = sb.tile([128, 1024], f32, tag="XY")
        H1 = sb.tile([128, 1024], f32, tag="H1")
        H2 = sb.tile([128, 1024], f32, tag="H2")
        ZU = sb.tile([128, 1024], f32, tag="ZU")
        SCA = sb.tile([128, 1024], f32, tag="SCA")
        SCB = sb.tile([128, 1024], f32, tag="SCB")
        SCR = sb.tile([128, 1024], f32, tag="SCR")
        lacc = sb.tile([128, 64], f32, tag="lacc")
        eacc = sb.tile([128, 8], f32, tag="eacc")
        nc.vector.memset(lacc[:, :], 0.0)
        nc.vector.memset(eacc[:, :], 0.0)
        nc.sync.dma_start(out=XY[:, :], in_=x0p_d[:, :])

        def wk(off, p):
            return ck[32 * p: 32 * p + 32, off: off + 32]

        def bias(col):
            return ck[:, col: col + 1]

        def mm_all(psum, pairs):
            """pairs: list of (lhsT_col_offset, rhs_tensor) accumulated."""
            for p in range(4):
                for q in range(2):
                    n = len(pairs)
                    for i, (off, rhs) in enumerate(pairs):
                        nc.tensor.matmul(
                            out=SL(psum, p, q),
                            lhsT=wk(off, p),
                            rhs=SL(rhs, p, q),
                            start=(i == 0),
                            stop=(i == n - 1),
                            tile_position=(32 * p, 32 * p),
                        )

        # ---- init: Y0 MLP ----
        ph = ps.tile([128, 1024], f32, tag="ph")
        mm_all(ph, [(C_WY1, XY)])
        nc.scalar.activation(out=H1[:, :], in_=ph[:, :], func=AF.Tanh,
                             bias=bias(C_BY1))
        ph = ps.tile([128, 1024], f32, tag="ph")
        mm_all(ph, [(C_WY2, H1)])
        nc.scalar.activation(out=H2[:, :], in_=ph[:, :], func=AF.Tanh,
                             bias=bias(C_BY2))
        pu = ps.tile([128, 1024], f32, tag="pu")
        mm_all(pu, [(C_WYP, XY), (C_WY3, H2)])
        nc.scalar.activation(out=XY[:, :], in_=pu[:, :], func=AF.Identity,
                             bias=bias(C_BYI))

        # ---- time steps ----
        dwt = None
        for t in range(t_steps):
            c, toff = divmod(t, CHUNK)
            if toff == 0:
                dwt = dwpool.tile([97, CHUNK * 2 * BK], f32, tag="dw")
                for p in range(4):
                    nc.sync.dma_start(
                        out=dwt[32 * p: 32 * p + 1, :],
                        in_=dwp_d[p: p + 1,
                                  c * CHUNK * 2 * BK: (c + 1) * CHUNK * 2 * BK])

            def dwrow(p, q):
                o = (toff * 2 + q) * BK
                return dwt[32 * p: 32 * p + 1, o: o + BK]

            pdwb = ps.tile([128, 1024], f32, tag="pdwb")
            for p in range(4):
                for q in range(2):
                    nc.tensor.matmul(out=SL(pdwb, p, q),
                                     lhsT=ck[32 * p: 32 * p + 1,
                                             C_ONES: C_ONES + 32],
                                     rhs=dwrow(p, q), start=True, stop=True,
                                     tile_position=(32 * p, 32 * p))

            ph = ps.tile([128, 1024], f32, tag="ph")
            mm_all(ph, [(C_W1, XY)])
            nc.scalar.activation(out=H1[:, :], in_=ph[:, :], func=AF.Tanh,
                                 bias=bias(C_B1 + t))
            ph = ps.tile([128, 1024], f32, tag="ph")
            mm_all(ph, [(C_W2, H1)])
            nc.scalar.activation(out=H2[:, :], in_=ph[:, :], func=AF.Tanh,
                                 bias=bias(C_B2))
            pzu = ps.tile([128, 1024], f32, tag="pzu")
            mm_all(pzu, [(C_W3, H2), (C_WZY, XY)])
            nc.scalar.activation(out=ZU[:, :], in_=pzu[:, :], func=AF.Identity,
                                 bias=bias(C_B3))
            nc.vector.tensor_tensor(out=SCA[:, :], in0=XY[:, :],
                                    in1=pdwb[:, :], op=OP.mult)
            nc.vector.tensor_tensor(out=SCB[:, :], in0=ZU[:, :],
                                    in1=pdwb[:, :], op=OP.mult)
            nc.vector.scalar_tensor_tensor(
                out=SCR[:, :], in0=ZU[:, :], scalar=float(_ct(t)),
                in1=ZU[:, :], op0=OP.mult, op1=OP.mult,
                accum_out=lacc[:, t: t + 1])
            pu = ps.tile([128, 1024], f32, tag="pu")
            for p in range(4):
                for q in range(2):
                    o = SL(pu, p, q)
                    nc.tensor.matmul(out=o, lhsT=wk(C_WUXY, p),
                                     rhs=SL(XY, p, q), start=True, stop=False,
                                     tile_position=(32 * p, 32 * p))
                    nc.tensor.matmul(out=o, lhsT=wk(C_WUZU, p),
                                     rhs=SL(ZU, p, q), start=False, stop=False,
                                     tile_position=(32 * p, 32 * p))
                    nc.tensor.matmul(out=o, lhsT=wk(C_WUSA, p),
                                     rhs=SL(SCA, p, q), start=False, stop=False,
                                     tile_position=(32 * p, 32 * p))
                    nc.tensor.matmul(out=o, lhsT=wk(C_WUSB, p),
                                     rhs=SL(SCB, p, q), start=False, stop=False,
                                     tile_position=(32 * p, 32 * p))
                    nc.tensor.matmul(out=o,
                                     lhsT=ck[32 * p: 32 * p + 1,
                                             C_SIG: C_SIG + 32],
                                     rhs=dwrow(p, q), start=False, stop=True,
                                     tile_position=(32 * p, 32 * p))
            nc.scalar.activation(out=XY[:, :], in_=pu[:, :], func=AF.Identity,
                                 bias=bias(C_BXY))

        # ---- final bsde loss ----
        pe = ps.tile([128, 1024], f32, tag="pu")
        mm_all(pe, [(C_WE, XY)])
        nc.scalar.activation(out=SCR[:, :], in_=pe[:, :], func=AF.Square,
                             accum_out=eacc[:, 0: 1])
        nc.sync.dma_start(out=lacc_d[:, :], in_=lacc[:, :])
        nc.sync.dma_start(out=eacc_d[:, :], in_=eacc[:, :])

    nc.compile()
    _BUILT[t_steps] = nc
    return nc


def kernel(**inputs):
    from concourse.bass_utils import run_bass_kernel_spmd

    img = pack_weights(inputs)
    X0 = np.asarray(inputs["X0"], F32)
    dw = np.asarray(inputs["dw"], F32)
    in_maps = []
    for k in range(NCORES):
        in_maps.append({
            "dwp": pack_dw(dw, k),
            "x0p": pack_x0(X0, k),
            "wpack": img,
        })
    nc = build(T)
    res = run_bass_kernel_spmd(nc, in_maps, core_ids=list(range(NCORES)))
    laccs = [r["out_lacc"] for r in res.results]
    eaccs = [r["out_eacc"] for r in res.results]
    return reduce_outputs(laccs, eaccs)


if __name__ == "__main__":
    print("module ok")


# revision 8
# speedup vs baseline: 3.9064x; 3.9064x over previous
"""Trainium2 Bass kernel for nn_DeepModel_70703751626759 (deep-BSDE forward sim).

v2: bf16 matmul paths; exact fp32 state kept in a persistent PSUM
accumulator (only increments flow through bf16); strip-rotated psum/sbuf
maps to spread PE sub-array load; dw rows pre-shifted host-side so the
broadcast matmuls land off the congested diagonal sub-arrays.

Data-parallel over 8 NeuronCores: 32768 samples -> 4096/core -> 8 column
blocks of 512 samples, block (p, q) at partitions [32p, 32p+32) x free
[512q, 512(q+1)). Slot content: state = [X(16); Y(16)], ZU = [u(8);
Zv(16); dH(8)]. t folded into layer-1 biases; dH recomposed through h2;
multiplicative noise via PE-broadcast dw rows + input scaling; losses
reduced on-device via accum_out.
"""

import sys
import numpy as np

if "/opt/trn_rl_repo" not in sys.path:
    sys.path.insert(0, "/opt/trn_rl_repo")

N = 16
M = 8
T = 50
DT = 0.01
GAMMA = 0.1
SIGMA = 0.2
TAU = 0.5
H = 10
BATCH = 32768
NCORES = 8
CB = BATCH // NCORES
NB = 8
BK = 512
CHUNK = 13
NCHUNK = 4
LC = CHUNK * 2 * BK          # per-chunk dw elements per group row
import os
P1OFF = int(os.environ.get("K_P1OFF", "0"))
P2OFF = int(os.environ.get("K_P2OFF", "0"))
GAOFF = int(os.environ.get("K_GAOFF", "0"))
GBOFF = int(os.environ.get("K_GBOFF", "0"))

F32 = np.float32
try:
    import ml_dtypes
    BF16 = ml_dtypes.bfloat16
except ImportError:          # pragma: no cover
    BF16 = np.float32

# bf16 consts image (ckb) column offsets
K_W1 = 0
K_W2 = 32
K_W3 = 64
K_WZY = 96
K_WDXY = 128
K_WDZU = 160
K_WDSA = 192
K_WDSB = 224
K_WE = 256
K_WY1 = 288
K_WY2 = 320
K_WY3 = 352
K_ONES = 384
K_SGR = 416
K_BY3V = 448
K_ONES512 = 480
K_COLS = 992

# fp32 consts image (ck) column offsets
C_WYP = 0
C_B1 = 32          # 64 cols
C_B2 = 96
C_B3 = 97
C_BY1 = 98
C_BY2 = 99
C_COLS = 128


def _ct(t):
    w = 1.0 if (t == 0 or t == T - 1) else 2.0
    return 0.5 * DT * w * TAU * TAU


def _weight_blocks(inp):
    A = np.asarray(inp["A"], F32)
    Bm = np.asarray(inp["Bmat"], F32)
    C = np.asarray(inp["Cmat"], F32)
    D = np.asarray(inp["Dmat"], F32)
    ZW1 = np.asarray(inp["Z_W1"], F32)
    ZW2 = np.asarray(inp["Z_W2"], F32)
    ZW3 = np.asarray(inp["Z_W3"], F32)
    PW1 = np.asarray(inp["phi_W1"], F32)
    PW2 = np.asarray(inp["phi_W2"], F32)
    PW3 = np.asarray(inp["phi_W3"], F32)
    YW1 = np.asarray(inp["Y0_W1"], F32)
    YW2 = np.asarray(inp["Y0_W2"], F32)
    YW3 = np.asarray(inp["Y0_W3"], F32)
    I16 = np.eye(16, dtype=F32)

    def blk():
        return np.zeros((32, 32), F32)

    W1 = blk()
    W1[0:16, 0:10] = ZW1[1:, :]
    W1[0:16, 10:20] = PW1[1:, :]
    W2 = blk()
    W2[0:10, 0:10] = ZW2
    W2[10:20, 10:20] = PW2
    W3 = blk()
    W3[10:20, 0:8] = PW3
    W3[0:10, 8:24] = ZW3
    W3[0:10, 24:32] = ZW3 @ D
    W3[10:20, 24:32] = PW3
    WZY = blk()
    WZY[16:32, 24:32] = Bm
    WDXY = blk()
    WDXY[0:16, 0:16] = DT * A.T
    WDXY[0:16, 16:32] = -DT * I16
    WDXY[16:32, 16:32] = -DT * A
    WDZU = blk()
    WDZU[0:8, 0:16] = DT * Bm.T
    WDZU[8:24, 16:32] = -DT * C
    WDSA = blk()
    WDSA[0:16, 0:16] = C.T
    WDSB = blk()
    WDSB[0:8, 0:16] = D.T
    WDSB[8:24, 16:32] = I16
    WE = blk()
    WE[0:16, 0:16] = -I16
    WE[16:32, 0:16] = I16
    WY1 = blk()
    WY1[0:16, 0:10] = YW1
    WY2 = blk()
    WY2[0:10, 0:10] = YW2
    WY3 = blk()
    WY3[0:10, 16:32] = YW3
    WYP = blk()
    WYP[0:16, 0:16] = I16
    SGR = np.zeros((2, 32), F32)      # K=2 lhsT: [dw-row; ones-row]
    SGR[0, 0:16] = SIGMA
    SGR[1, 0:16] = GAMMA * DT
    BY3V = np.zeros((1, 32), F32)
    BY3V[0, 16:32] = np.asarray(inp["Y0_b3"], F32)
    return dict(W1=W1, W2=W2, W3=W3, WZY=WZY, WDXY=WDXY, WDZU=WDZU,
                WDSA=WDSA, WDSB=WDSB, WE=WE, WY1=WY1, WY2=WY2, WY3=WY3,
                WYP=WYP, SGR=SGR, BY3V=BY3V)


def pack_weights_bf16(inp):
    wb = _weight_blocks(inp)
    img = np.zeros((128, K_COLS), F32)
    reps = [(K_W1, "W1"), (K_W2, "W2"), (K_W3, "W3"), (K_WZY, "WZY"),
            (K_WDXY, "WDXY"), (K_WDZU, "WDZU"), (K_WDSA, "WDSA"),
            (K_WDSB, "WDSB"), (K_WE, "WE"), (K_WY1, "WY1"),
            (K_WY2, "WY2"), (K_WY3, "WY3")]
    for p in range(4):
        r = 32 * p
        for off, name in reps:
            img[r: r + 32, off: off + 32] = wb[name]
        img[r: r + 1, K_BY3V: K_BY3V + 32] = wb["BY3V"]
        img[r: r + 2, K_SGR: K_SGR + 32] = wb["SGR"]
    img[:, K_ONES: K_ONES + 32] = 1.0
    img[:, K_ONES512: K_ONES512 + 512] = 1.0
    return img.astype(BF16)


def pack_weights_f32(inp):
    wb = _weight_blocks(inp)
    Zb1 = np.asarray(inp["Z_b1"], F32)
    Zb2 = np.asarray(inp["Z_b2"], F32)
    Zb3 = np.asarray(inp["Z_b3"], F32)
    Pb1 = np.asarray(inp["phi_b1"], F32)
    Pb2 = np.asarray(inp["phi_b2"], F32)
    Pb3 = np.asarray(inp["phi_b3"], F32)
    Yb1 = np.asarray(inp["Y0_b1"], F32)
    Yb2 = np.asarray(inp["Y0_b2"], F32)
    ZW1 = np.asarray(inp["Z_W1"], F32)
    PW1 = np.asarray(inp["phi_W1"], F32)
    D = np.asarray(inp["Dmat"], F32)
    img = np.zeros((128, C_COLS), F32)
    for t in range(T):
        tv = F32(t * DT)
        b = np.concatenate([Zb1 + tv * ZW1[0, :], Pb1 + tv * PW1[0, :]])
        for p in range(4):
            img[32 * p: 32 * p + 20, C_B1 + t] = b
    b2 = np.concatenate([Zb2, Pb2])
    b3 = np.concatenate([Pb3, Zb3, Pb3 + D.T @ Zb3])
    for p in range(4):
        r = 32 * p
        img[r: r + 32, C_WYP: C_WYP + 32] = wb["WYP"]
        img[r: r + 20, C_B2] = b2
        img[r: r + 32, C_B3] = b3
        img[r: r + 10, C_BY1] = Yb1
        img[r: r + 10, C_BY2] = Yb2
    return img


def pack_x0(X0, core):
    out = np.zeros((128, 1024), F32)
    base = core * CB
    for p in range(4):
        for q in range(2):
            j = p + 4 * q
            out[32 * p: 32 * p + 16, 512 * q: 512 * (q + 1)] = \
                X0[base + BK * j: base + BK * (j + 1), :].T
    return np.ascontiguousarray(out)


def pack_dw(dw, core):
    """v1 layout (group g holds block g) for the emulator."""
    base = core * CB
    out = np.zeros((4, NCHUNK * LC), F32)
    for c in range(NCHUNK):
        for toff in range(CHUNK):
            t = c * CHUNK + toff
            if t >= T:
                break
            for g in range(4):
                for q in range(2):
                    j = g + 4 * q
                    o = c * LC + (toff * 2 + q) * BK
                    out[g, o: o + BK] = dw[t, base + BK * j: base + BK * (j + 1), 0]
    return np.ascontiguousarray(out)


def pack_dw_dev(dw, core):
    """Device layouts: dwa[g] holds block (g+3)%4 (+4q); dwb rows
    (2g, 2g+1) hold [block (g+2)%4 dw; ones]."""
    base = core * CB
    dwa = np.zeros((4, NCHUNK * LC), F32)
    dwb = np.zeros((8, NCHUNK * LC), F32)
    dwb[1::2, :] = 1.0
    for c in range(NCHUNK):
        for toff in range(CHUNK):
            t = c * CHUNK + toff
            if t >= T:
                break
            for g in range(4):
                for q in range(2):
                    o = c * LC + (toff * 2 + q) * BK
                    ja = (g + 4 - GAOFF) % 4 + 4 * q
                    jb = (g + 4 - GBOFF) % 4 + 4 * q
                    dwa[g, o: o + BK] = dw[t, base + BK * ja: base + BK * ja + BK, 0]
                    dwb[2 * g, o: o + BK] = dw[t, base + BK * jb: base + BK * jb + BK, 0]
    return dwa.astype(BF16), dwb.astype(BF16)


# ---------------------------------------------------------------------------
# numpy emulation (exact fp32 algebra; validates packing + math)
# ---------------------------------------------------------------------------

def emulate_core(inp, core, t_steps=T):
    wb = _weight_blocks(inp)
    ckf = pack_weights_f32(inp)
    x0p = pack_x0(np.asarray(inp["X0"], F32), core)
    dwp = pack_dw(np.asarray(inp["dw"], F32), core)

    def slots(x):
        return [x[32 * (j % 4): 32 * (j % 4) + 32,
                  512 * (j // 4): 512 * (j // 4) + 512] for j in range(NB)]

    def bias(col):
        return ckf[:, col: col + 1]

    lacc = np.zeros((128, 64), F32)
    eacc = np.zeros((128, 8), F32)

    ST = np.zeros((128, 1024), F32)       # P_state
    for j in range(NB):
        slots(ST)[j][:] = wb["WYP"].T @ slots(x0p)[j]
    XY = ST.copy()
    ph = np.zeros_like(ST)
    for j in range(NB):
        slots(ph)[j][:] = wb["WY1"].T @ slots(XY)[j]
    H1 = np.tanh(ph + bias(C_BY1))
    for j in range(NB):
        slots(ph)[j][:] = wb["WY2"].T @ slots(H1)[j]
    H2 = np.tanh(ph + bias(C_BY2))
    ones_row = np.ones((1, BK), F32)
    for j in range(NB):
        slots(ST)[j][:] += (wb["WY3"].T @ slots(H2)[j]
                            + wb["BY3V"].T @ ones_row)

    for t in range(t_steps):
        c, toff = divmod(t, CHUNK)
        XY = ST.copy()
        pdwb = np.zeros_like(ST)
        for j in range(NB):
            g, q = j % 4, j // 4
            o = c * LC + (toff * 2 + q) * BK
            slots(pdwb)[j][:] = np.broadcast_to(dwp[g, o: o + BK], (32, BK))
        for j in range(NB):
            slots(ph)[j][:] = wb["W1"].T @ slots(XY)[j]
        H1 = np.tanh(ph + bias(C_B1 + t))
        for j in range(NB):
            slots(ph)[j][:] = wb["W2"].T @ slots(H1)[j]
        H2 = np.tanh(ph + bias(C_B2))
        pzu = np.zeros_like(ST)
        for j in range(NB):
            slots(pzu)[j][:] = (wb["W3"].T @ slots(H2)[j]
                                + wb["WZY"].T @ slots(XY)[j])
        ZU = pzu + bias(C_B3)
        SCA = XY * pdwb
        SCB = ZU * pdwb
        lacc[:, t: t + 1] = np.sum((_ct(t) * ZU) * ZU, axis=1, keepdims=True)
        for j in range(NB):
            g, q = j % 4, j // 4
            o = c * LC + (toff * 2 + q) * BK
            dwrow = dwp[g: g + 1, o: o + BK]
            rhs_sg = np.concatenate([dwrow, ones_row], axis=0)
            slots(ST)[j][:] += (wb["WDXY"].T @ slots(XY)[j]
                                + wb["WDZU"].T @ slots(ZU)[j]
                                + wb["WDSA"].T @ slots(SCA)[j]
                                + wb["WDSB"].T @ slots(SCB)[j]
                                + wb["SGR"].T @ rhs_sg)
    XY = ST.copy()
    pe = np.zeros_like(ST)
    for j in range(NB):
        slots(pe)[j][:] = wb["WE"].T @ slots(XY)[j]
    eacc[:, 0: 1] = np.sum(pe * pe, axis=1, keepdims=True)
    return lacc, eacc


def reduce_outputs(laccs, eaccs, t_steps=T):
    dh_rows = np.zeros(128, bool)
    e_rows = np.zeros(128, bool)
    for p in range(4):
        dh_rows[32 * p + 24: 32 * p + 32] = True
        e_rows[32 * p: 32 * p + 16] = True
    lc = 0.0
    lb = 0.0
    for lacc, eacc in zip(laccs, eaccs):
        lc += float(np.sum(np.asarray(lacc, np.float64)[dh_rows, :t_steps]))
        lb += float(np.sum(np.asarray(eacc, np.float64)[e_rows, 0]))
    return np.array([lb / BATCH, lc / BATCH], F32)


# ---------------------------------------------------------------------------
# device program
# ---------------------------------------------------------------------------

_BUILT = {}


def build(t_steps=T):
    if t_steps in _BUILT:
        return _BUILT[t_steps]
    from contextlib import ExitStack
    import concourse.tile as tile
    from concourse import bacc, mybir

    f32 = mybir.dt.float32
    bf16 = mybir.dt.bfloat16
    AF = mybir.ActivationFunctionType
    OP = mybir.AluOpType

    nc = bacc.Bacc("TRN2", target_bir_lowering=False, debug=False)
    dwa_d = nc.dram_tensor("dwa", [4, NCHUNK * LC], bf16,
                           kind="ExternalInput").ap()
    dwb_d = nc.dram_tensor("dwb", [8, NCHUNK * LC], bf16,
                           kind="ExternalInput").ap()
    x0p_d = nc.dram_tensor("x0p", [128, 1024], f32, kind="ExternalInput").ap()
    ckb_d = nc.dram_tensor("ckb", [128, K_COLS], bf16,
                           kind="ExternalInput").ap()
    ckf_d = nc.dram_tensor("ckf", [128, C_COLS], f32,
                           kind="ExternalInput").ap()
    lacc_d = nc.dram_tensor("out_lacc", [128, 64], f32,
                            kind="ExternalOutput").ap()
    eacc_d = nc.dram_tensor("out_eacc", [128, 8], f32,
                            kind="ExternalOutput").ap()

    def SL(tens, g, q):
        return tens[32 * g: 32 * g + 32, 512 * q: 512 * (q + 1)]

    with tile.TileContext(nc) as tc, ExitStack() as ctx:
        sb = ctx.enter_context(tc.tile_pool(name="sb", bufs=1))
        dwpool = ctx.enter_context(tc.tile_pool(name="dwp", bufs=2))
        ps = ctx.enter_context(tc.tile_pool(name="ps", bufs=1, space="PSUM"))

        ckb = sb.tile([128, K_COLS], bf16, tag="ckb")
        ckf = sb.tile([128, C_COLS], f32, tag="ckf")
        nc.sync.dma_start(out=ckb[:, :], in_=ckb_d[:, :])
        nc.sync.dma_start(out=ckf[:, :], in_=ckf_d[:, :])
        X0SB = sb.tile([128, 1024], f32, tag="X0SB")
        nc.sync.dma_start(out=X0SB[:, :], in_=x0p_d[:, :])
        XY = sb.tile([128, 1024], bf16, tag="XY")
        H1 = sb.tile([128, 1024], bf16, tag="H1")
        H2 = sb.tile([128, 1024], bf16, tag="H2")
        ZU = sb.tile([128, 1024], bf16, tag="ZU")
        SCA = sb.tile([128, 1024], bf16, tag="SCA")
        SCB = sb.tile([128, 1024], bf16, tag="SCB")
        SCR = sb.tile([128, 1024], bf16, tag="SCR")
        lacc = sb.tile([128, 64], f32, tag="lacc")
        eacc = sb.tile([128, 8], f32, tag="eacc")
        nc.vector.memset(lacc[:, :], 0.0)
        nc.vector.memset(eacc[:, :], 0.0)

        PST = ps.tile([128, 1024], f32, tag="pst")   # persistent state

        def wkb(off, g):
            return ckb[32 * g: 32 * g + 32, off: off + 32]

        def wkf(off, g):
            return ckf[32 * g: 32 * g + 32, off: off + 32]

        def bias(col):
            return ckf[:, col: col + 1]

        def mm(out_t, og, oq, lhsT, rhs, start, stop, rg):
            nc.tensor.matmul(out=SL(out_t, og, oq), lhsT=lhsT, rhs=rhs,
                             start=start, stop=stop,
                             tile_position=(32 * rg, 32 * og),
                             skip_group_check=True)

        # ---- init ----
        for p in range(4):
            for q in range(2):
                mm(PST, p, q, wkf(C_WYP, p), SL(X0SB, p, q), True, False, p)
        nc.scalar.activation(out=XY[:, :], in_=PST[:, :], func=AF.Copy)
        ph = ps.tile([128, 1024], f32, tag="ph")
        for p in range(4):
            for q in range(2):
                p1 = (p + P1OFF) % 4
                mm(ph, p1, q, wkb(K_WY1, p), SL(XY, p, q), True, True, p)
        nc.scalar.activation(out=H1[:, :], in_=ph[:, :], func=AF.Tanh,
                             bias=bias(C_BY1))
        ph = ps.tile([128, 1024], f32, tag="ph")
        for p in range(4):
            for q in range(2):
                p1, p2 = (p + P1OFF) % 4, (p + P2OFF) % 4
                mm(ph, p2, q, wkb(K_WY2, p1), SL(H1, p1, q), True, True, p1)
        nc.scalar.activation(out=H2[:, :], in_=ph[:, :], func=AF.Tanh,
                             bias=bias(C_BY2))
        for p in range(4):
            for q in range(2):
                p2, p3 = (p + P2OFF) % 4, (p + P2OFF) % 4
                mm(PST, p, q, wkb(K_WY3, p2), SL(H2, p2, q), False, False, p2)
                mm(PST, p, q, ckb[32 * p3: 32 * p3 + 1, K_BY3V: K_BY3V + 32],
                   ckb[32 * p3: 32 * p3 + 1, K_ONES512: K_ONES512 + 512],
                   False, False, p3)

        # ---- steps ----
        dwat = dwbt = None
        for t in range(t_steps):
            c, toff = divmod(t, CHUNK)
            if toff == 0:
                dwat = dwpool.tile([97, LC], bf16, tag="dwa")
                dwbt = dwpool.tile([98, LC], bf16, tag="dwb")
                for g in range(4):
                    nc.sync.dma_start(out=dwat[32 * g: 32 * g + 1, :],
                                      in_=dwa_d[g: g + 1, c * LC: (c + 1) * LC])
                    nc.sync.dma_start(out=dwbt[32 * g: 32 * g + 2, :],
                                      in_=dwb_d[2 * g: 2 * g + 2,
                                                c * LC: (c + 1) * LC])

            o = (toff * 2)
            nc.scalar.activation(out=XY[:, :], in_=PST[:, :], func=AF.Copy)

            pdwb = ps.tile([128, 1024], f32, tag="pdwb")
            for p in range(4):
                for q in range(2):
                    ga = (p + GAOFF) % 4
                    mm(pdwb, p, q,
                       ckb[32 * ga: 32 * ga + 1, K_ONES: K_ONES + 32],
                       dwat[32 * ga: 32 * ga + 1, (o + q) * BK: (o + q + 1) * BK],
                       True, True, ga)

            ph = ps.tile([128, 1024], f32, tag="ph")
            for p in range(4):
                for q in range(2):
                    p1 = (p + P1OFF) % 4
                    mm(ph, p1, q, wkb(K_W1, p), SL(XY, p, q), True, True, p)
            nc.scalar.activation(out=H1[:, :], in_=ph[:, :], func=AF.Tanh,
                                 bias=bias(C_B1 + t))
            ph = ps.tile([128, 1024], f32, tag="ph")
            for p in range(4):
                for q in range(2):
                    p1, p2 = (p + P1OFF) % 4, (p + P2OFF) % 4
                    mm(ph, p2, q, wkb(K_W2, p1), SL(H1, p1, q), True, True, p1)
            nc.scalar.activation(out=H2[:, :], in_=ph[:, :], func=AF.Tanh,
                                 bias=bias(C_B2))
            pzu = ps.tile([128, 1024], f32, tag="pzu")
            for p in range(4):
                for q in range(2):
                    p2 = (p + P2OFF) % 4
                    mm(pzu, p, q, wkb(K_W3, p2), SL(H2, p2, q), True, False, p2)
                    mm(pzu, p, q, wkb(K_WZY, p), SL(XY, p, q), False, True, p)
            nc.scalar.activation(out=ZU[:, :], in_=pzu[:, :], func=AF.Identity,
                                 bias=bias(C_B3))
            nc.vector.tensor_tensor(out=SCA[:, :], in0=XY[:, :],
                                    in1=pdwb[:, :], op=OP.mult)
            nc.vector.tensor_tensor(out=SCB[:, :], in0=ZU[:, :],
                                    in1=pdwb[:, :], op=OP.mult)
            nc.vector.scalar_tensor_tensor(
                out=SCR[:, :], in0=ZU[:, :], scalar=float(_ct(t)),
                in1=ZU[:, :], op0=OP.mult, op1=OP.mult,
                accum_out=lacc[:, t: t + 1])
            last = (t == t_steps - 1)
            for p in range(4):
                for q in range(2):
                    gb = (p + GBOFF) % 4
                    mm(PST, p, q, wkb(K_WDXY, p), SL(XY, p, q), False, False, p)
                    mm(PST, p, q, wkb(K_WDZU, p), SL(ZU, p, q), False, False, p)
                    mm(PST, p, q, wkb(K_WDSA, p), SL(SCA, p, q), False, False, p)
                    mm(PST, p, q, wkb(K_WDSB, p), SL(SCB, p, q), False, False, p)
                    # sigma/gamma: K=2 rhs [dw; ones] at rows 32gb..+2
                    nc.tensor.matmul(
                        out=SL(PST, p, q),
                        lhsT=ckb[32 * gb: 32 * gb + 2, K_SGR: K_SGR + 32],
                        rhs=dwbt[32 * gb: 32 * gb + 2,
                                 (o + q) * BK: (o + q + 1) * BK],
                        start=False, stop=(last and p == 3 and q == 1),
                        tile_position=(32 * gb, 32 * p),
                        skip_group_check=True)

        # ---- final ----
        nc.scalar.activation(out=XY[:, :], in_=PST[:, :], func=AF.Copy)
        pe = ps.tile([128, 1024], f32, tag="ph")
        for p in range(4):
            for q in range(2):
                mm(pe, p, q, wkb(K_WE, p), SL(XY, p, q), True, True, p)
        nc.scalar.activation(out=SCR[:, :], in_=pe[:, :], func=AF.Square,
                             accum_out=eacc[:, 0: 1])
        nc.sync.dma_start(out=lacc_d[:, :], in_=lacc[:, :])
        nc.sync.dma_start(out=eacc_d[:, :], in_=eacc[:, :])

    nc.compile()
    _BUILT[t_steps] = nc
    return nc


def kernel(**inputs):
    from concourse.bass_utils import run_bass_kernel_spmd

    ckb = pack_weights_bf16(inputs)
    ckf = pack_weights_f32(inputs)
    X0 = np.asarray(inputs["X0"], F32)
    dw = np.asarray(inputs["dw"], F32)
    in_maps = []
    for k in range(NCORES):
        dwa, dwb = pack_dw_dev(dw, k)
        in_maps.append({
            "dwa": dwa,
            "dwb": dwb,
            "x0p": pack_x0(X0, k),
            "ckb": ckb,
            "ckf": ckf,
        })
    nc = build(T)
    res = run_bass_kernel_spmd(nc, in_maps, core_ids=list(range(NCORES)))
    laccs = [r["out_lacc"] for r in res.results]
    eaccs = [r["out_eacc"] for r in res.results]
    return reduce_outputs(laccs, eaccs)


if __name__ == "__main__":
    print("module ok")
